# revision 53
# baseline (speedup 1.0000x reference)
"""Wide-gather variant.

Algebra: s = (s1+s2)/2 collapses to s = g[e0] + g[e1] + b_edge with
g[n] = relu(emb[n] @ W_emb + b_emb) . (w1+w2)/2, so the kernel is a
per-node scalar table (50K f32) plus 2 table lookups per edge.

Per core (200704 edge slots incl pad): the g table lives in SBUF as
[128, 3136] f32 where partition 16G+c holds g block c (node ids are
host-remapped to n' = 6272*(n//6250) + n%6250 so each core's AllGather
contribution is one contiguous block).  Each 28672-edge supertile's
57344 lookups run as two GPSIMD ap_gather calls (4096- and 3072-wide
per group; the ucode op has no 1024-element dst limit, unlike the
native indirect_copy ISA instruction) -- out width ~ table width, the
per-call cost floor.  Host
pre-computes the uint16 word index and the bf16 block-id field, so
there is no on-chip index arithmetic: per 4096-edge chunk it's two PE
broadcast matmuls, two DVE is_equal*mult ops, and two PE reduce
matmuls (float32r, 512 cols) into an accumulating PSUM bank, plus one
noise-term add matmul.  Gate math: 2xLn on [56,512] supertile tiles
(ACT), t1-t2 on DVE, sigmoid per chunk into a [56,512] staging tile,
one output DMA per supertile.  Noise/output are host-permuted;
embedding is loaded bf16.
"""
import sys
sys.path.insert(0, '/opt/trn_rl_repo')
import numpy as np

N, IN_DIM, HID = 50000, 256, 64
E = 1_600_000
BIAS = 0.0001
NCORES = 8
EC = E // NCORES            # 200000 edges per core
NNC = N // NCORES           # 6250 nodes per core
NNCP = 6272                 # padded (16*392), so NALL = 8*6272
NALL = NCORES * NNCP        # 50176 = 16 * 3136
TABW = NALL // 16           # 3136 table words per partition
CH = 512                    # chunk columns (one PSUM bank)
NCH = 7                     # chunk-pairs per supertile
WO = NCH * CH               # 3584 = per-stream width
WO2 = 2 * WO                # 7168 = lookups per group per supertile
ST = 8 * WO                 # 28672 edges per supertile
NST = 7                     # supertiles per core
ECP = NST * ST              # 200704 padded edge slots
IDXF = WO2 // 16            # 448 idx columns
SPLITS = ((0, 4), (4, 3))   # (first chunk-pair, n chunk-pairs) per gather

_nc = None
_maps = None
_REPS = 1   # benchmarking knob: repeat phase 2 this many times

# const-pack column offsets ([128, CBW] f32)
CB_BD = 0            # [128, 56]   cols 24:32 = bdiag8, rest 0 (sliding lhsT)
CB_IOTA = 56         # [128, 1]    iota16
CB_BEMB = 57         # [64, 1]
CB_WBAR = 58         # [64, 1]
CB_BHALF = 59        # [1, 1]
CB_I56 = 60          # [56, 56]
CB_A1 = 116
CB_B1 = 117
CB_A2 = 118
CB_B2 = 119
CB_BE = 120
CBW = 121


def _build():
    from concourse import bass, bacc, tile, mybir

    f32 = mybir.dt.float32
    f32r = mybir.dt.float32r
    bf16 = mybir.dt.bfloat16
    i16 = mybir.dt.int16
    ACT = mybir.ActivationFunctionType
    OP = mybir.AluOpType
    nc = bacc.Bacc("TRN2", target_bir_lowering=False, debug=False,
                   num_devices=NCORES)

    embT = nc.dram_tensor("embT", [128, 2, NNCP], bf16, kind="ExternalInput")
    wi_d = nc.dram_tensor("wi", [NST, 128, IDXF], i16, kind="ExternalInput")
    cf_d = nc.dram_tensor("cf", [NST, 8, WO2], bf16, kind="ExternalInput")
    nz_d = nc.dram_tensor("nz", [NST, 56, CH], f32, kind="ExternalInput")
    cb_d = nc.dram_tensor("cb", [128, CBW + 128], f32, kind="ExternalInput")
    cbr_d = nc.dram_tensor("cbr", [128, 169], f32r, kind="ExternalInput")
    out_d = nc.dram_tensor("out", [NST, 56, CH], f32, kind="ExternalOutput")

    def r(ap):
        return ap.bitcast(f32r)

    with tile.TileContext(nc) as tc:
        with tc.tile_pool(name="const", bufs=1) as cp, \
             tc.tile_pool(name="tab", bufs=1) as tabp, \
             tc.tile_pool(name="dram", bufs=1, space="DRAM") as dram:
            cb = cp.tile([128, CBW + 128], f32, tag="cb")
            nc.sync.dma_start(out=cb[:], in_=cb_d[:, :])
            w01 = cb[:, CBW:CBW + 64].bitcast(bf16)
            ex8 = cb[0:8, CBW + 64:CBW + 128].bitcast(bf16)
            cbr = cp.tile([128, 169], f32r, tag="cbr")
            nc.sync.dma_start(out=cbr[:], in_=cbr_d[:, :])
            bd = cbr[:, 0:56]
            i56r = cbr[0:56, 56:112]
            wbarr = cbr[0:HID, 112:113]
            i56n = cbr[0:56, 113:169]
            iota16 = cb[:, CB_IOTA:CB_IOTA + 1]
            bemb = cb[0:HID, CB_BEMB:CB_BEMB + 1]
            wbar = cb[0:HID, CB_WBAR:CB_WBAR + 1]
            bhalf = cb[0:1, CB_BHALF:CB_BHALF + 1]
            i56 = cb[0:56, CB_I56:CB_I56 + 56]
            a1t = cb[0:56, CB_A1:CB_A1 + 1]
            b1t = cb[0:56, CB_B1:CB_B1 + 1]
            a2t = cb[0:56, CB_A2:CB_A2 + 1]
            b2t = cb[0:56, CB_B2:CB_B2 + 1]
            bet = cb[0:56, CB_BE:CB_BE + 1]

            # ---------- phase 1: per-node scalars g ----------
            g_sb = cp.tile([1, NNCP], f32, tag="gsb")
            with tc.tile_pool(name="p1", bufs=4) as p1, \
                 tc.tile_pool(name="ps1", bufs=2, space="PSUM") as ps1, \
                 tc.tile_pool(name="ps1g", bufs=2, space="PSUM") as ps1g:
                col = 0
                while col < NNCP:
                    n = min(1024, NNCP - col)
                    e01 = p1.tile([128, 2 * n], bf16, tag="e01")
                    nc.sync.dma_start(
                        out=e01[:].rearrange("p (two m) -> p two m", two=2),
                        in_=embT[:, :, col:col + n])
                    ph = ps1.tile([HID, n], f32, tag="ph")
                    for sub in range(0, n, 512):
                        m = min(512, n - sub)
                        nc.tensor.matmul(out=ph[:, sub:sub + m],
                                         lhsT=w01[:, 0:HID],
                                         rhs=e01[:, sub:sub + m],
                                         start=True, stop=False)
                        nc.tensor.matmul(out=ph[:, sub:sub + m],
                                         lhsT=w01[:, HID:2 * HID],
                                         rhs=e01[:, n + sub:n + sub + m],
                                         start=False, stop=True)
                    hT = p1.tile([HID, n], f32r, tag="hT")
                    nc.scalar.activation(out=hT[:], in_=ph[:],
                                         func=ACT.Relu, bias=bemb)
                    pg = ps1g.tile([1, n], f32, tag="pg")
                    for sub in range(0, n, 512):
                        m = min(512, n - sub)
                        nc.tensor.matmul(out=pg[:, sub:sub + m], lhsT=wbarr,
                                         rhs=hT[:, sub:sub + m],
                                         start=True, stop=True)
                    nc.vector.tensor_scalar(out=g_sb[0:1, col:col + n],
                                            in0=pg[:], scalar1=bhalf,
                                            scalar2=None, op0=OP.add)
                    col += n

            g_mine = dram.tile([1, NNCP], f32, tag="gmine")
            g_all = dram.tile([1, NALL], f32, tag="gall")
            nc.sync.dma_start(out=g_mine[:], in_=g_sb[0:1, :])
            nc.gpsimd.collective_compute(
                "AllGather", bass.mybir.AluOpType.bypass,
                replica_groups=[list(range(NCORES))],
                ins=[g_mine[:].opt()], outs=[g_all[:].opt()])

            # table[16G+c, w] = g_all[3136*c + w]: contiguous per partition
            table = tabp.tile([128, TABW], f32, tag="table")
            g_all_cw = g_all[0].rearrange("(c w) -> c w", c=16)
            for G in range(8):
                eng = nc.sync if G < 4 else nc.gpsimd
                eng.dma_start(out=table[16 * G:16 * G + 16, :],
                              in_=g_all_cw)

            # ---------------- phase 2: edges ----------------
            with tc.tile_pool(name="idx", bufs=2) as idxp, \
                 tc.tile_pool(name="cfp", bufs=2) as cfp, \
                 tc.tile_pool(name="cnd", bufs=3) as cndp, \
                 tc.tile_pool(name="msk", bufs=3) as mskp, \
                 tc.tile_pool(name="nzp", bufs=2) as nzp, \
                 tc.tile_pool(name="gate", bufs=2) as gatep, \
                 tc.tile_pool(name="outp", bufs=2) as outp, \
                 tc.tile_pool(name="psc", bufs=3, space="PSUM") as psc, \
                 tc.tile_pool(name="pss", bufs=1, space="PSUM") as pss:
                for t in [tt for _ in range(_REPS) for tt in range(NST)]:
                    wi = idxp.tile([128, IDXF], i16, tag="wi")
                    nc.sync.dma_start(out=wi[:], in_=wi_d[t])
                    cf = cfp.tile([8, WO2], bf16, tag="cf")
                    nc.sync.dma_start(out=cf[:], in_=cf_d[t])
                    cns = []
                    wcol = ccol = 0
                    for si, (k0, nk) in enumerate(SPLITS):
                        w = 2 * nk * CH
                        cn = cndp.tile([128, w], f32, tag=f"cn{si}")
                        nc.gpsimd.ap_gather(
                            out_ap=cn[:], in_ap=table[:],
                            idxs_ap=wi[:, wcol:wcol + w // 16],
                            channels=128, num_elems=TABW, d=1, num_idxs=w)
                        cns.append((cn, ccol, nk))
                        wcol += w // 16
                        ccol += w

                    nz = nzp.tile([56, CH], f32, tag="nz")
                    nc.sync.dma_start(out=nz[:], in_=nz_d[t])
                    t1 = gatep.tile([56, CH], f32, tag="t1")
                    nc.scalar.activation(out=t1[:], in_=nz[:], func=ACT.Ln,
                                         bias=b1t, scale=a1t)
                    t2 = gatep.tile([56, CH], f32, tag="t2")
                    nc.scalar.activation(out=t2[:], in_=nz[:], func=ACT.Ln,
                                         bias=b2t, scale=a2t)
                    td = gatep.tile([56, CH], f32r, tag="td")
                    nc.vector.scalar_tensor_tensor(
                        out=td[:], in0=t1[:], scalar=0.0, in1=t2[:],
                        op0=OP.add, op1=OP.subtract)

                    # per split si: one PSUM tile [8*nk, CH]; reduce matmul
                    # for local chunk j uses the sliding-window lhsT
                    # bd[:, 32-8j : 32-8j+8*nk] (zeros outside row band 8j,
                    # harmless under accumulation).  One add matmul + one
                    # sigmoid per split.
                    ot = outp.tile([56, CH], f32, tag="ot")
                    for si, (k0, nk) in enumerate(SPLITS):
                        cn, ccol, _ = cns[si]
                        ps_s = pss.tile([8 * nk, CH], f32, tag=f"ps{si}")
                        first = True
                        for s in range(2):
                            sb = nk * CH * s
                            for cl0 in range(0, nk * CH, 2 * CH):
                                w = min(2 * CH, nk * CH - cl0)
                                ps_c = psc.tile([128, 2 * CH], f32,
                                                tag="psc")
                                for o in range(0, w, CH):
                                    nc.tensor.matmul(
                                        out=ps_c[:, o:o + CH], lhsT=ex8,
                                        rhs=cf[:, ccol + sb + cl0 + o:
                                               ccol + sb + cl0 + o + CH],
                                        start=True, stop=True)
                                msk = mskp.tile([128, 2 * CH], f32r,
                                                tag="msk")
                                nc.vector.scalar_tensor_tensor(
                                    out=msk[:, 0:w], in0=ps_c[:, 0:w],
                                    scalar=iota16,
                                    in1=cn[:, sb + cl0:sb + cl0 + w],
                                    op0=OP.is_equal, op1=OP.mult)
                                for o in range(0, w, CH):
                                    j = (cl0 + o) // CH
                                    nc.tensor.matmul(
                                        out=ps_s[:], rhs=msk[:, o:o + CH],
                                        lhsT=bd[:, 24 - 8 * j:
                                                24 - 8 * j + 8 * nk],
                                        start=first, stop=False,
                                        skip_group_check=True)
                                    first = False
                        nc.tensor.matmul(
                            out=ps_s[:], rhs=td[:],
                            lhsT=i56r[:, 8 * k0:8 * k0 + 8 * nk],
                            start=False, stop=True, skip_group_check=True)
                        nc.scalar.activation(
                            out=ot[8 * k0:8 * k0 + 8 * nk, :], in_=ps_s[:],
                            func=ACT.Sigmoid)
                    nc.sync.dma_start(out=out_d[t], in_=ot[:])
    nc.compile()
    return nc


def _get_nc():
    global _nc
    if _nc is None:
        _nc = _build()
    return _nc


def _get_maps():
    """Per-core slot mappings (identical for every core), computed once."""
    global _maps
    if _maps is None:
        lid = np.arange(EC)
        t = lid // ST
        pos = lid % ST
        f = pos // 128
        rr = pos % 128
        g8 = rr // 16
        cs = rr % 16
        j = f * 16 + cs
        k = j // CH
        q = j % CH
        nidx = t * ST + (8 * k + g8) * CH + q       # into [NST, 56, CH] flat
        # lookup column within the supertile's gather calls, per stream:
        # call A (chunk-pairs 0..3): src 512k+q, dst 2048+512k+q
        # call B (chunk-pairs 4..6): src 512(k-4)+q, dst 1536+512(k-4)+q
        inA = k < 4
        iA = np.where(inA, CH * k + q, CH * (k - 4) + q)
        span = np.where(inA, 4 * CH, 3 * CH)        # per-stream width of call
        cbase = np.where(inA, 0, 8 * CH)            # call base in cf cols
        wbase = np.where(inA, 0, CH // 2)           # call base in wi cols
        _maps = (nidx, t, rr, g8, iA, span, cbase, wbase)
    return _maps


def _stream_idx(s):
    """(widx, cidx) flat positions for stream s in wi [NST,128,IDXF] and
    cf [NST,8,WO2]."""
    nidx, t, rr, g8, iA, span, cbase, wbase = _get_maps()
    i = iA + span * s
    cidx = (t * 8 + g8) * WO2 + cbase + i
    widx = (t * 128 + rr) * IDXF + wbase + i // 16
    return widx, cidx


def prepare_in_maps(embedding, edges, noise, W_emb, b_emb, W_edge, b_edge):
    import ml_dtypes
    embedding = np.asarray(embedding, dtype=np.float32)
    edges = np.asarray(edges)
    noise = np.asarray(noise, dtype=np.float32)
    W_emb = np.asarray(W_emb, dtype=np.float32)
    b_emb = np.asarray(b_emb, dtype=np.float32)
    W_edge = np.asarray(W_edge, dtype=np.float32)
    b_edge = np.float32(b_edge)

    bf = ml_dtypes.bfloat16
    wbar = ((W_edge[:HID] + W_edge[HID:]) * 0.5).astype(np.float32)
    # w01[p, s*HID+h] = W_emb[s*128+p, h]
    w01 = np.ascontiguousarray(
        W_emb.reshape(2, 128, HID).transpose(1, 0, 2).reshape(128, 2 * HID)
    ).astype(bf)
    p = np.arange(128)
    ex8 = (p[None, :] // 16 == np.arange(8)[:, None]).astype(bf)

    a1, b1 = 2.0 * BIAS - 1.0, 1.0 - BIAS
    a2, b2 = 1.0 - 2.0 * BIAS, BIAS
    cb = np.zeros((128, CBW + 128), dtype=np.float32)
    cb[:, CBW:CBW + 64] = np.ascontiguousarray(w01).view(np.float32)
    cb[0:8, CBW + 64:CBW + 128] = np.ascontiguousarray(ex8).view(np.float32)
    cb[:, CB_BD + 24:CB_BD + 32] = (p[:, None] // 16 ==
                                    np.arange(8)[None, :]).astype(np.float32)
    cb[:, CB_IOTA] = (p % 16).astype(np.float32)
    cb[0:HID, CB_BEMB] = b_emb
    cb[0:HID, CB_WBAR] = wbar
    cb[0, CB_BHALF] = b_edge * 0.5
    cb[0:56, CB_I56:CB_I56 + 56] = np.eye(56, dtype=np.float32)
    cb[0:56, CB_A1] = a1
    cb[0:56, CB_B1] = b1
    cb[0:56, CB_A2] = a2
    cb[0:56, CB_B2] = b2

    cbr = np.zeros((128, 169), dtype=np.float32)
    cbr[0:56, 113:169] = -np.eye(56, dtype=np.float32)
    cbr[:, 0:56] = cb[:, CB_BD:CB_BD + 56]
    cbr[0:56, 56:112] = cb[0:56, CB_I56:CB_I56 + 56]
    cbr[0:HID, 112] = wbar

    nidx = _get_maps()[0]
    sidx = [_stream_idx(0), _stream_idx(1)]

    in_maps = []
    for core in range(NCORES):
        sl = embedding[core * NNC:(core + 1) * NNC]
        embT = np.zeros((128, 2, NNCP), dtype=bf)
        embT[:, 0, :NNC] = sl.T[:128].astype(bf)
        embT[:, 1, :NNC] = sl.T[128:].astype(bf)

        wi = np.zeros(NST * 128 * IDXF, dtype=np.int16)
        cf = np.zeros(NST * 8 * WO2, dtype=np.float32)
        for s in range(2):
            n = edges[s, core * EC:(core + 1) * EC].astype(np.int64)
            kk = n // NNC
            npr = NNCP * kk + (n - NNC * kk)
            c = npr // TABW
            w = npr - TABW * c
            widx, cidx = sidx[s]
            wi[widx] = w.astype(np.int16)
            cf[cidx] = c.astype(np.float32)
        nz = np.full(NST * 56 * CH, 0.5, dtype=np.float32)
        nz[nidx] = noise[core * EC:(core + 1) * EC]

        in_maps.append({
            "embT": embT,
            "wi": wi.reshape(NST, 128, IDXF),
            "cf": cf.reshape(NST, 8, WO2).astype(bf),
            "nz": nz.reshape(NST, 56, CH),
            "cb": cb, "cbr": cbr,
        })
    return in_maps


def kernel(embedding, edges, noise, W_emb, b_emb, W_edge, b_edge):
    from concourse import bass_utils
    nc = _get_nc()
    in_maps = prepare_in_maps(embedding, edges, noise, W_emb, b_emb,
                              W_edge, b_edge)
    res = bass_utils.run_bass_kernel_spmd(nc, in_maps,
                                          core_ids=list(range(NCORES)))
    nidx = _get_maps()[0]
    out = np.empty(E, dtype=np.float32)
    for core in range(NCORES):
        out[core * EC:(core + 1) * EC] = \
            res.results[core]["out"].reshape(-1)[nidx]
    return out


# revision 60
# speedup vs baseline: 1.0148x; 1.0148x over previous
"""Wide-gather variant.

Algebra: s = (s1+s2)/2 collapses to s = g[e0] + g[e1] + b_edge with
g[n] = relu(emb[n] @ W_emb + b_emb) . (w1+w2)/2, so the kernel is a
per-node scalar table (50K f32) plus 2 table lookups per edge.

Per core (200704 edge slots incl pad): the g table lives in SBUF as
[128, 3136] f32 where partition 16G+c holds g block c (node ids are
host-remapped to n' = 6272*(n//6250) + n%6250 so each core's AllGather
contribution is one contiguous block).  Each 28672-edge supertile's
57344 lookups run as two GPSIMD ap_gather calls (4096- and 3072-wide
per group; the ucode op has no 1024-element dst limit, unlike the
native indirect_copy ISA instruction) -- out width ~ table width, the
per-call cost floor.  Host
pre-computes the uint16 word index and the bf16 block-id field, so
there is no on-chip index arithmetic: per 4096-edge chunk it's two PE
broadcast matmuls, two DVE is_equal*mult ops, and two PE reduce
matmuls (float32r, 512 cols) into an accumulating PSUM bank, plus one
noise-term add matmul.  Gate math: 2xLn on [56,512] supertile tiles
(ACT), t1-t2 on DVE, sigmoid per chunk into a [56,512] staging tile,
one output DMA per supertile.  Noise/output are host-permuted;
embedding is loaded bf16.
"""
import sys
sys.path.insert(0, '/opt/trn_rl_repo')
import numpy as np

N, IN_DIM, HID = 50000, 256, 64
E = 1_600_000
BIAS = 0.0001
NCORES = 8
EC = E // NCORES            # 200000 edges per core
NNC = N // NCORES           # 6250 nodes per core
NNCP = 6272                 # padded (16*392), so NALL = 8*6272
NALL = NCORES * NNCP        # 50176 = 16 * 3136
TABW = NALL // 16           # 3136 table words per partition
CH = 512                    # chunk columns (one PSUM bank)
NCH = 7                     # chunk-pairs per supertile
WO = NCH * CH               # 3584 = per-stream width
WO2 = 2 * WO                # 7168 = lookups per group per supertile
ST = 8 * WO                 # 28672 edges per supertile
NST = 7                     # supertiles per core
ECP = NST * ST              # 200704 padded edge slots
IDXF = WO2 // 16            # 448 idx columns
SPLITS = ((0, 4), (4, 3))   # (first chunk-pair, n chunk-pairs) per gather

_nc = None
_maps = None
_REPS = 1   # benchmarking knob: repeat phase 2 this many times

# const-pack column offsets ([128, CBW] f32)
CB_BD = 0            # [128, 56]   cols 24:32 = bdiag8, rest 0 (sliding lhsT)
CB_IOTA = 56         # [128, 1]    iota16
CB_BEMB = 57         # [64, 1]
CB_WBAR = 58         # [64, 1]
CB_BHALF = 59        # [1, 1]
CB_I56 = 60          # [56, 56]
CB_A1 = 116
CB_B1 = 117
CB_A2 = 118
CB_B2 = 119
CB_BE = 120
CBW = 121


def _build():
    from concourse import bass, bacc, tile, mybir

    f32 = mybir.dt.float32
    f32r = mybir.dt.float32r
    bf16 = mybir.dt.bfloat16
    i16 = mybir.dt.int16
    ACT = mybir.ActivationFunctionType
    OP = mybir.AluOpType
    nc = bacc.Bacc("TRN2", target_bir_lowering=False, debug=False,
                   num_devices=NCORES)

    embT = nc.dram_tensor("embT", [128, 2, NNCP], bf16, kind="ExternalInput")
    wi_d = nc.dram_tensor("wi", [NST, 128, IDXF], i16, kind="ExternalInput")
    cf_d = nc.dram_tensor("cf", [NST, 8, WO2], bf16, kind="ExternalInput")
    nz_d = nc.dram_tensor("nz", [NST, 56, CH], f32, kind="ExternalInput")
    cb_d = nc.dram_tensor("cb", [128, CBW + 128], f32, kind="ExternalInput")
    cbr_d = nc.dram_tensor("cbr", [128, 169], f32r, kind="ExternalInput")
    out_d = nc.dram_tensor("out", [NST, 56, CH], f32, kind="ExternalOutput")

    def r(ap):
        return ap.bitcast(f32r)

    with tile.TileContext(nc) as tc:
        with tc.tile_pool(name="const", bufs=1) as cp, \
             tc.tile_pool(name="tab", bufs=1) as tabp, \
             tc.tile_pool(name="dram", bufs=1, space="DRAM") as dram:
            cb = cp.tile([128, CBW + 128], f32, tag="cb")
            nc.sync.dma_start(out=cb[:], in_=cb_d[:, :])
            w01 = cb[:, CBW:CBW + 64].bitcast(bf16)
            ex8 = cb[0:8, CBW + 64:CBW + 128].bitcast(bf16)
            cbr = cp.tile([128, 169], f32r, tag="cbr")
            nc.sync.dma_start(out=cbr[:], in_=cbr_d[:, :])
            bd = cbr[:, 0:56]
            i56r = cbr[0:56, 56:112]
            wbarr = cbr[0:HID, 112:113]
            i56n = cbr[0:56, 113:169]
            iota16 = cb[:, CB_IOTA:CB_IOTA + 1]
            bemb = cb[0:HID, CB_BEMB:CB_BEMB + 1]
            wbar = cb[0:HID, CB_WBAR:CB_WBAR + 1]
            bhalf = cb[0:1, CB_BHALF:CB_BHALF + 1]
            i56 = cb[0:56, CB_I56:CB_I56 + 56]
            a1t = cb[0:56, CB_A1:CB_A1 + 1]
            b1t = cb[0:56, CB_B1:CB_B1 + 1]
            a2t = cb[0:56, CB_A2:CB_A2 + 1]
            b2t = cb[0:56, CB_B2:CB_B2 + 1]
            bet = cb[0:56, CB_BE:CB_BE + 1]

            # ---------- phase 1: per-node scalars g ----------
            g_sb = cp.tile([1, NNCP], f32, tag="gsb")
            with tc.tile_pool(name="p1", bufs=4) as p1, \
                 tc.tile_pool(name="ps1", bufs=4, space="PSUM") as ps1, \
                 tc.tile_pool(name="ps1g", bufs=4, space="PSUM") as ps1g:
                col = 0
                while col < NNCP:
                    n = min(1024, NNCP - col)
                    e01 = p1.tile([128, 2 * n], bf16, tag="e01")
                    nc.sync.dma_start(
                        out=e01[:].rearrange("p (two m) -> p two m", two=2),
                        in_=embT[:, :, col:col + n])
                    for sub in range(0, n, 512):
                        m = min(512, n - sub)
                        ph = ps1.tile([HID, 512], f32, tag="ph")
                        nc.tensor.matmul(out=ph[:, 0:m],
                                         lhsT=w01[:, 0:HID],
                                         rhs=e01[:, sub:sub + m],
                                         start=True, stop=False)
                        nc.tensor.matmul(out=ph[:, 0:m],
                                         lhsT=w01[:, HID:2 * HID],
                                         rhs=e01[:, n + sub:n + sub + m],
                                         start=False, stop=True)
                        hT = p1.tile([HID, 512], f32r, tag="hT")
                        nc.scalar.activation(out=hT[:, 0:m], in_=ph[:, 0:m],
                                             func=ACT.Relu, bias=bemb)
                        pg = ps1g.tile([1, 512], f32, tag="pg")
                        nc.tensor.matmul(out=pg[:, 0:m], lhsT=wbarr,
                                         rhs=hT[:, 0:m],
                                         start=True, stop=True)
                        nc.vector.tensor_scalar(
                            out=g_sb[0:1, col + sub:col + sub + m],
                            in0=pg[:, 0:m], scalar1=bhalf,
                            scalar2=None, op0=OP.add)
                    col += n

            g_mine = dram.tile([1, NNCP], f32, tag="gmine")
            g_all = dram.tile([1, NALL], f32, tag="gall")
            nc.sync.dma_start(out=g_mine[:], in_=g_sb[0:1, :])
            nc.gpsimd.collective_compute(
                "AllGather", bass.mybir.AluOpType.bypass,
                replica_groups=[list(range(NCORES))],
                ins=[g_mine[:].opt()], outs=[g_all[:].opt()])

            # table[16G+c, w] = g_all[3136*c + w]: contiguous per partition
            table = tabp.tile([128, TABW], f32, tag="table")
            g_all_cw = g_all[0].rearrange("(c w) -> c w", c=16)
            for G in range(8):
                eng = nc.sync if G < 8 else nc.gpsimd
                eng.dma_start(out=table[16 * G:16 * G + 16, :],
                              in_=g_all_cw)

            # ---------------- phase 2: edges ----------------
            with tc.tile_pool(name="idx", bufs=2) as idxp, \
                 tc.tile_pool(name="cfp", bufs=2) as cfp, \
                 tc.tile_pool(name="cnd", bufs=3) as cndp, \
                 tc.tile_pool(name="msk", bufs=3) as mskp, \
                 tc.tile_pool(name="nzp", bufs=2) as nzp, \
                 tc.tile_pool(name="gate", bufs=2) as gatep, \
                 tc.tile_pool(name="outp", bufs=2) as outp, \
                 tc.tile_pool(name="psc", bufs=3, space="PSUM") as psc, \
                 tc.tile_pool(name="pss", bufs=1, space="PSUM") as pss:
                for t in [tt for _ in range(_REPS) for tt in range(NST)]:
                    wi = idxp.tile([128, IDXF], i16, tag="wi")
                    nc.sync.dma_start(out=wi[:], in_=wi_d[t])
                    cf = cfp.tile([8, WO2], bf16, tag="cf")
                    nc.sync.dma_start(out=cf[:], in_=cf_d[t])
                    cns = []
                    wcol = ccol = 0
                    for si, (k0, nk) in enumerate(SPLITS):
                        w = 2 * nk * CH
                        cn = cndp.tile([128, w], f32, tag=f"cn{si}")
                        nc.gpsimd.ap_gather(
                            out_ap=cn[:], in_ap=table[:],
                            idxs_ap=wi[:, wcol:wcol + w // 16],
                            channels=128, num_elems=TABW, d=1, num_idxs=w)
                        cns.append((cn, ccol, nk))
                        wcol += w // 16
                        ccol += w

                    nz = nzp.tile([56, CH], f32, tag="nz")
                    nc.sync.dma_start(out=nz[:], in_=nz_d[t])
                    t1 = gatep.tile([56, CH], f32, tag="t1")
                    nc.scalar.activation(out=t1[:], in_=nz[:], func=ACT.Ln,
                                         bias=b1t, scale=a1t)
                    t2 = gatep.tile([56, CH], f32, tag="t2")
                    nc.scalar.activation(out=t2[:], in_=nz[:], func=ACT.Ln,
                                         bias=b2t, scale=a2t)
                    td = gatep.tile([56, CH], f32r, tag="td")
                    nc.vector.scalar_tensor_tensor(
                        out=td[:], in0=t1[:], scalar=0.0, in1=t2[:],
                        op0=OP.add, op1=OP.subtract)

                    # per split si: one PSUM tile [8*nk, CH]; reduce matmul
                    # for local chunk j uses the sliding-window lhsT
                    # bd[:, 32-8j : 32-8j+8*nk] (zeros outside row band 8j,
                    # harmless under accumulation).  One add matmul + one
                    # sigmoid per split.
                    ot = outp.tile([56, CH], f32, tag="ot")
                    for si, (k0, nk) in enumerate(SPLITS):
                        cn, ccol, _ = cns[si]
                        ps_s = pss.tile([8 * nk, CH], f32, tag=f"ps{si}")
                        first = True
                        for s in range(2):
                            sb = nk * CH * s
                            for cl0 in range(0, nk * CH, 2 * CH):
                                w = min(2 * CH, nk * CH - cl0)
                                ps_c = psc.tile([128, 2 * CH], f32,
                                                tag="psc")
                                for o in range(0, w, CH):
                                    nc.tensor.matmul(
                                        out=ps_c[:, o:o + CH], lhsT=ex8,
                                        rhs=cf[:, ccol + sb + cl0 + o:
                                               ccol + sb + cl0 + o + CH],
                                        start=True, stop=True)
                                msk = mskp.tile([128, 2 * CH], f32r,
                                                tag="msk")
                                nc.vector.scalar_tensor_tensor(
                                    out=msk[:, 0:w], in0=ps_c[:, 0:w],
                                    scalar=iota16,
                                    in1=cn[:, sb + cl0:sb + cl0 + w],
                                    op0=OP.is_equal, op1=OP.mult)
                                for o in range(0, w, CH):
                                    j = (cl0 + o) // CH
                                    nc.tensor.matmul(
                                        out=ps_s[:], rhs=msk[:, o:o + CH],
                                        lhsT=bd[:, 24 - 8 * j:
                                                24 - 8 * j + 8 * nk],
                                        start=first, stop=False,
                                        skip_group_check=True)
                                    first = False
                        nc.tensor.matmul(
                            out=ps_s[:], rhs=td[:],
                            lhsT=i56r[:, 8 * k0:8 * k0 + 8 * nk],
                            start=False, stop=True, skip_group_check=True)
                        nc.scalar.activation(
                            out=ot[8 * k0:8 * k0 + 8 * nk, :], in_=ps_s[:],
                            func=ACT.Sigmoid)
                    nc.sync.dma_start(out=out_d[t], in_=ot[:])
    nc.compile()
    return nc


def _get_nc():
    global _nc
    if _nc is None:
        _nc = _build()
    return _nc


def _get_maps():
    """Per-core slot mappings (identical for every core), computed once."""
    global _maps
    if _maps is None:
        lid = np.arange(EC)
        t = lid // ST
        pos = lid % ST
        f = pos // 128
        rr = pos % 128
        g8 = rr // 16
        cs = rr % 16
        j = f * 16 + cs
        k = j // CH
        q = j % CH
        nidx = t * ST + (8 * k + g8) * CH + q       # into [NST, 56, CH] flat
        # lookup column within the supertile's gather calls, per stream:
        # call A (chunk-pairs 0..3): src 512k+q, dst 2048+512k+q
        # call B (chunk-pairs 4..6): src 512(k-4)+q, dst 1536+512(k-4)+q
        inA = k < 4
        iA = np.where(inA, CH * k + q, CH * (k - 4) + q)
        span = np.where(inA, 4 * CH, 3 * CH)        # per-stream width of call
        cbase = np.where(inA, 0, 8 * CH)            # call base in cf cols
        wbase = np.where(inA, 0, CH // 2)           # call base in wi cols
        _maps = (nidx, t, rr, g8, iA, span, cbase, wbase)
    return _maps


def _stream_idx(s):
    """(widx, cidx) flat positions for stream s in wi [NST,128,IDXF] and
    cf [NST,8,WO2]."""
    nidx, t, rr, g8, iA, span, cbase, wbase = _get_maps()
    i = iA + span * s
    cidx = (t * 8 + g8) * WO2 + cbase + i
    widx = (t * 128 + rr) * IDXF + wbase + i // 16
    return widx, cidx


def prepare_in_maps(embedding, edges, noise, W_emb, b_emb, W_edge, b_edge):
    import ml_dtypes
    embedding = np.asarray(embedding, dtype=np.float32)
    edges = np.asarray(edges)
    noise = np.asarray(noise, dtype=np.float32)
    W_emb = np.asarray(W_emb, dtype=np.float32)
    b_emb = np.asarray(b_emb, dtype=np.float32)
    W_edge = np.asarray(W_edge, dtype=np.float32)
    b_edge = np.float32(b_edge)

    bf = ml_dtypes.bfloat16
    wbar = ((W_edge[:HID] + W_edge[HID:]) * 0.5).astype(np.float32)
    # w01[p, s*HID+h] = W_emb[s*128+p, h]
    w01 = np.ascontiguousarray(
        W_emb.reshape(2, 128, HID).transpose(1, 0, 2).reshape(128, 2 * HID)
    ).astype(bf)
    p = np.arange(128)
    ex8 = (p[None, :] // 16 == np.arange(8)[:, None]).astype(bf)

    a1, b1 = 2.0 * BIAS - 1.0, 1.0 - BIAS
    a2, b2 = 1.0 - 2.0 * BIAS, BIAS
    cb = np.zeros((128, CBW + 128), dtype=np.float32)
    cb[:, CBW:CBW + 64] = np.ascontiguousarray(w01).view(np.float32)
    cb[0:8, CBW + 64:CBW + 128] = np.ascontiguousarray(ex8).view(np.float32)
    cb[:, CB_BD + 24:CB_BD + 32] = (p[:, None] // 16 ==
                                    np.arange(8)[None, :]).astype(np.float32)
    cb[:, CB_IOTA] = (p % 16).astype(np.float32)
    cb[0:HID, CB_BEMB] = b_emb
    cb[0:HID, CB_WBAR] = wbar
    cb[0, CB_BHALF] = b_edge * 0.5
    cb[0:56, CB_I56:CB_I56 + 56] = np.eye(56, dtype=np.float32)
    cb[0:56, CB_A1] = a1
    cb[0:56, CB_B1] = b1
    cb[0:56, CB_A2] = a2
    cb[0:56, CB_B2] = b2

    cbr = np.zeros((128, 169), dtype=np.float32)
    cbr[0:56, 113:169] = -np.eye(56, dtype=np.float32)
    cbr[:, 0:56] = cb[:, CB_BD:CB_BD + 56]
    cbr[0:56, 56:112] = cb[0:56, CB_I56:CB_I56 + 56]
    cbr[0:HID, 112] = wbar

    nidx = _get_maps()[0]
    sidx = [_stream_idx(0), _stream_idx(1)]

    in_maps = []
    for core in range(NCORES):
        sl = embedding[core * NNC:(core + 1) * NNC]
        embT = np.zeros((128, 2, NNCP), dtype=bf)
        embT[:, 0, :NNC] = sl.T[:128].astype(bf)
        embT[:, 1, :NNC] = sl.T[128:].astype(bf)

        wi = np.zeros(NST * 128 * IDXF, dtype=np.int16)
        cf = np.zeros(NST * 8 * WO2, dtype=np.float32)
        for s in range(2):
            n = edges[s, core * EC:(core + 1) * EC].astype(np.int64)
            kk = n // NNC
            npr = NNCP * kk + (n - NNC * kk)
            c = npr // TABW
            w = npr - TABW * c
            widx, cidx = sidx[s]
            wi[widx] = w.astype(np.int16)
            cf[cidx] = c.astype(np.float32)
        nz = np.full(NST * 56 * CH, 0.5, dtype=np.float32)
        nz[nidx] = noise[core * EC:(core + 1) * EC]

        in_maps.append({
            "embT": embT,
            "wi": wi.reshape(NST, 128, IDXF),
            "cf": cf.reshape(NST, 8, WO2).astype(bf),
            "nz": nz.reshape(NST, 56, CH),
            "cb": cb, "cbr": cbr,
        })
    return in_maps


def kernel(embedding, edges, noise, W_emb, b_emb, W_edge, b_edge):
    from concourse import bass_utils
    nc = _get_nc()
    in_maps = prepare_in_maps(embedding, edges, noise, W_emb, b_emb,
                              W_edge, b_edge)
    res = bass_utils.run_bass_kernel_spmd(nc, in_maps,
                                          core_ids=list(range(NCORES)))
    nidx = _get_maps()[0]
    out = np.empty(E, dtype=np.float32)
    for core in range(NCORES):
        out[core * EC:(core + 1) * EC] = \
            res.results[core]["out"].reshape(-1)[nidx]
    return out


# revision 61
# speedup vs baseline: 1.0195x; 1.0047x over previous
"""Wide-gather variant.

Algebra: s = (s1+s2)/2 collapses to s = g[e0] + g[e1] + b_edge with
g[n] = relu(emb[n] @ W_emb + b_emb) . (w1+w2)/2, so the kernel is a
per-node scalar table (50K f32) plus 2 table lookups per edge.

Per core (200704 edge slots incl pad): the g table lives in SBUF as
[128, 3136] f32 where partition 16G+c holds g block c (node ids are
host-remapped to n' = 6272*(n//6250) + n%6250 so each core's AllGather
contribution is one contiguous block).  Each 28672-edge supertile's
57344 lookups run as two GPSIMD ap_gather calls (4096- and 3072-wide
per group; the ucode op has no 1024-element dst limit, unlike the
native indirect_copy ISA instruction) -- out width ~ table width, the
per-call cost floor.  Host
pre-computes the uint16 word index and the bf16 block-id field, so
there is no on-chip index arithmetic: per 4096-edge chunk it's two PE
broadcast matmuls, two DVE is_equal*mult ops, and two PE reduce
matmuls (float32r, 512 cols) into an accumulating PSUM bank, plus one
noise-term add matmul.  Gate math: 2xLn on [56,512] supertile tiles
(ACT), t1-t2 on DVE, sigmoid per chunk into a [56,512] staging tile,
one output DMA per supertile.  Noise/output are host-permuted;
embedding is loaded bf16.
"""
import sys
sys.path.insert(0, '/opt/trn_rl_repo')
import numpy as np

N, IN_DIM, HID = 50000, 256, 64
E = 1_600_000
BIAS = 0.0001
NCORES = 8
EC = E // NCORES            # 200000 edges per core
NNC = N // NCORES           # 6250 nodes per core
NNCP = 6272                 # padded (16*392), so NALL = 8*6272
NALL = NCORES * NNCP        # 50176 = 16 * 3136
TABW = NALL // 16           # 3136 table words per partition
CH = 512                    # chunk columns (one PSUM bank)
NCH = 7                     # chunk-pairs per supertile
WO = NCH * CH               # 3584 = per-stream width
WO2 = 2 * WO                # 7168 = lookups per group per supertile
ST = 8 * WO                 # 28672 edges per supertile
NST = 7                     # supertiles per core
ECP = NST * ST              # 200704 padded edge slots
IDXF = WO2 // 16            # 448 idx columns
SPLITS = ((0, 4), (4, 3))   # (first chunk-pair, n chunk-pairs) per gather

_nc = None
_maps = None
_REPS = 1   # benchmarking knob: repeat phase 2 this many times

# const-pack column offsets ([128, CBW] f32)
CB_BD = 0            # [128, 56]   cols 24:32 = bdiag8, rest 0 (sliding lhsT)
CB_IOTA = 56         # [128, 1]    iota16
CB_BEMB = 57         # [64, 1]
CB_WBAR = 58         # [64, 1]
CB_BHALF = 59        # [1, 1]
CB_I56 = 60          # [56, 56]
CB_A1 = 116
CB_B1 = 117
CB_A2 = 118
CB_B2 = 119
CB_BE = 120
CBW = 121


def _build():
    from concourse import bass, bacc, tile, mybir

    f32 = mybir.dt.float32
    f32r = mybir.dt.float32r
    bf16 = mybir.dt.bfloat16
    i16 = mybir.dt.int16
    ACT = mybir.ActivationFunctionType
    OP = mybir.AluOpType
    nc = bacc.Bacc("TRN2", target_bir_lowering=False, debug=False,
                   num_devices=NCORES)

    embT = nc.dram_tensor("embT", [128, 2, NNCP], bf16, kind="ExternalInput")
    wi_d = nc.dram_tensor("wi", [NST, 128, IDXF], i16, kind="ExternalInput")
    cf_d = nc.dram_tensor("cf", [NST, 8, WO2], bf16, kind="ExternalInput")
    nz_d = nc.dram_tensor("nz", [NST, 56, CH], f32, kind="ExternalInput")
    cb_d = nc.dram_tensor("cb", [128, CBW + 128], f32, kind="ExternalInput")
    cbr_d = nc.dram_tensor("cbr", [128, 169], f32r, kind="ExternalInput")
    out_d = nc.dram_tensor("out", [NST, 56, CH], f32, kind="ExternalOutput")

    def r(ap):
        return ap.bitcast(f32r)

    with tile.TileContext(nc) as tc:
        with tc.tile_pool(name="const", bufs=1) as cp, \
             tc.tile_pool(name="tab", bufs=1) as tabp, \
             tc.tile_pool(name="dram", bufs=1, space="DRAM") as dram:
            cb = cp.tile([128, CBW + 128], f32, tag="cb")
            nc.sync.dma_start(out=cb[:], in_=cb_d[:, :])
            w01 = cb[:, CBW:CBW + 64].bitcast(bf16)
            ex8 = cb[0:8, CBW + 64:CBW + 128].bitcast(bf16)
            cbr = cp.tile([128, 169], f32r, tag="cbr")
            nc.sync.dma_start(out=cbr[:], in_=cbr_d[:, :])
            bd = cbr[:, 0:56]
            i56r = cbr[0:56, 56:112]
            wbarr = cbr[0:HID, 112:113]
            i56n = cbr[0:56, 113:169]
            iota16 = cb[:, CB_IOTA:CB_IOTA + 1]
            bemb = cb[0:HID, CB_BEMB:CB_BEMB + 1]
            wbar = cb[0:HID, CB_WBAR:CB_WBAR + 1]
            bhalf = cb[0:1, CB_BHALF:CB_BHALF + 1]
            i56 = cb[0:56, CB_I56:CB_I56 + 56]
            a1t = cb[0:56, CB_A1:CB_A1 + 1]
            b1t = cb[0:56, CB_B1:CB_B1 + 1]
            a2t = cb[0:56, CB_A2:CB_A2 + 1]
            b2t = cb[0:56, CB_B2:CB_B2 + 1]
            bet = cb[0:56, CB_BE:CB_BE + 1]

            # ---------- phase 1: per-node scalars g ----------
            g_sb = cp.tile([1, NNCP], f32, tag="gsb")
            with tc.tile_pool(name="p1", bufs=4) as p1, \
                 tc.tile_pool(name="ps1", bufs=4, space="PSUM") as ps1, \
                 tc.tile_pool(name="ps1g", bufs=4, space="PSUM") as ps1g:
                col = 0
                while col < NNCP:
                    n = min(1024, NNCP - col)
                    e01 = p1.tile([128, 2 * n], bf16, tag="e01")
                    nc.sync.dma_start(
                        out=e01[:].rearrange("p (two m) -> p two m", two=2),
                        in_=embT[:, :, col:col + n])
                    for sub in range(0, n, 512):
                        m = min(512, n - sub)
                        ph = ps1.tile([HID, 512], f32, tag="ph")
                        nc.tensor.matmul(out=ph[:, 0:m],
                                         lhsT=w01[:, 0:HID],
                                         rhs=e01[:, sub:sub + m],
                                         start=True, stop=False)
                        nc.tensor.matmul(out=ph[:, 0:m],
                                         lhsT=w01[:, HID:2 * HID],
                                         rhs=e01[:, n + sub:n + sub + m],
                                         start=False, stop=True)
                        hT = p1.tile([HID, 512], f32r, tag="hT")
                        nc.scalar.activation(out=hT[:, 0:m], in_=ph[:, 0:m],
                                             func=ACT.Relu, bias=bemb)
                        pg = ps1g.tile([1, 512], f32, tag="pg")
                        nc.tensor.matmul(out=pg[:, 0:m], lhsT=wbarr,
                                         rhs=hT[:, 0:m],
                                         start=True, stop=True)
                        nc.vector.tensor_scalar(
                            out=g_sb[0:1, col + sub:col + sub + m],
                            in0=pg[:, 0:m], scalar1=bhalf,
                            scalar2=None, op0=OP.add)
                    col += n

            g_mine = dram.tile([1, NNCP], f32, tag="gmine")
            g_all = dram.tile([1, NALL], f32, tag="gall")
            nc.sync.dma_start(out=g_mine[:], in_=g_sb[0:1, :])
            nc.gpsimd.collective_compute(
                "AllGather", bass.mybir.AluOpType.bypass,
                replica_groups=[list(range(NCORES))],
                ins=[g_mine[:].opt()], outs=[g_all[:].opt()])

            # table[16G+c, w] = g_all[3136*c + w]: contiguous per partition
            table = tabp.tile([128, TABW], f32, tag="table")
            g_all_cw = g_all[0].rearrange("(c w) -> c w", c=16)
            for G in range(8):
                eng = nc.sync if G < 7 else nc.gpsimd
                eng.dma_start(out=table[16 * G:16 * G + 16, :],
                              in_=g_all_cw)

            # ---------------- phase 2: edges ----------------
            with tc.tile_pool(name="idx", bufs=2) as idxp, \
                 tc.tile_pool(name="cfp", bufs=2) as cfp, \
                 tc.tile_pool(name="cnd", bufs=3) as cndp, \
                 tc.tile_pool(name="msk", bufs=3) as mskp, \
                 tc.tile_pool(name="nzp", bufs=2) as nzp, \
                 tc.tile_pool(name="gate", bufs=2) as gatep, \
                 tc.tile_pool(name="outp", bufs=2) as outp, \
                 tc.tile_pool(name="psc", bufs=3, space="PSUM") as psc, \
                 tc.tile_pool(name="pss", bufs=1, space="PSUM") as pss:
                for t in [tt for _ in range(_REPS) for tt in range(NST)]:
                    wi = idxp.tile([128, IDXF], i16, tag="wi")
                    nc.sync.dma_start(out=wi[:], in_=wi_d[t])
                    cf = cfp.tile([8, WO2], bf16, tag="cf")
                    nc.sync.dma_start(out=cf[:], in_=cf_d[t])
                    cns = []
                    wcol = ccol = 0
                    for si, (k0, nk) in enumerate(SPLITS):
                        w = 2 * nk * CH
                        cn = cndp.tile([128, w], f32, tag=f"cn{si}")
                        nc.gpsimd.ap_gather(
                            out_ap=cn[:], in_ap=table[:],
                            idxs_ap=wi[:, wcol:wcol + w // 16],
                            channels=128, num_elems=TABW, d=1, num_idxs=w)
                        cns.append((cn, ccol, nk))
                        wcol += w // 16
                        ccol += w

                    nz = nzp.tile([56, CH], f32, tag="nz")
                    nc.sync.dma_start(out=nz[:], in_=nz_d[t])
                    t1 = gatep.tile([56, CH], f32, tag="t1")
                    nc.scalar.activation(out=t1[:], in_=nz[:], func=ACT.Ln,
                                         bias=b1t, scale=a1t)
                    t2 = gatep.tile([56, CH], f32, tag="t2")
                    nc.scalar.activation(out=t2[:], in_=nz[:], func=ACT.Ln,
                                         bias=b2t, scale=a2t)
                    td = gatep.tile([56, CH], f32r, tag="td")
                    nc.vector.scalar_tensor_tensor(
                        out=td[:], in0=t1[:], scalar=0.0, in1=t2[:],
                        op0=OP.add, op1=OP.subtract)

                    # per split si: one PSUM tile [8*nk, CH]; reduce matmul
                    # for local chunk j uses the sliding-window lhsT
                    # bd[:, 32-8j : 32-8j+8*nk] (zeros outside row band 8j,
                    # harmless under accumulation).  One add matmul + one
                    # sigmoid per split.
                    ot = outp.tile([56, CH], f32, tag="ot")
                    for si, (k0, nk) in enumerate(SPLITS):
                        cn, ccol, _ = cns[si]
                        ps_s = pss.tile([8 * nk, CH], f32, tag=f"ps{si}")
                        first = True
                        for s in range(2):
                            sb = nk * CH * s
                            for cl0 in range(0, nk * CH, 2 * CH):
                                w = min(2 * CH, nk * CH - cl0)
                                ps_c = psc.tile([128, 2 * CH], f32,
                                                tag="psc")
                                for o in range(0, w, CH):
                                    nc.tensor.matmul(
                                        out=ps_c[:, o:o + CH], lhsT=ex8,
                                        rhs=cf[:, ccol + sb + cl0 + o:
                                               ccol + sb + cl0 + o + CH],
                                        start=True, stop=True)
                                msk = mskp.tile([128, 2 * CH], f32r,
                                                tag="msk")
                                nc.vector.scalar_tensor_tensor(
                                    out=msk[:, 0:w], in0=ps_c[:, 0:w],
                                    scalar=iota16,
                                    in1=cn[:, sb + cl0:sb + cl0 + w],
                                    op0=OP.is_equal, op1=OP.mult)
                                for o in range(0, w, CH):
                                    j = (cl0 + o) // CH
                                    nc.tensor.matmul(
                                        out=ps_s[:], rhs=msk[:, o:o + CH],
                                        lhsT=bd[:, 24 - 8 * j:
                                                24 - 8 * j + 8 * nk],
                                        start=first, stop=False,
                                        skip_group_check=True)
                                    first = False
                        nc.tensor.matmul(
                            out=ps_s[:], rhs=td[:],
                            lhsT=i56r[:, 8 * k0:8 * k0 + 8 * nk],
                            start=False, stop=True, skip_group_check=True)
                        nc.scalar.activation(
                            out=ot[8 * k0:8 * k0 + 8 * nk, :], in_=ps_s[:],
                            func=ACT.Sigmoid)
                    nc.sync.dma_start(out=out_d[t], in_=ot[:])
    nc.compile()
    return nc


def _get_nc():
    global _nc
    if _nc is None:
        _nc = _build()
    return _nc


def _get_maps():
    """Per-core slot mappings (identical for every core), computed once."""
    global _maps
    if _maps is None:
        lid = np.arange(EC)
        t = lid // ST
        pos = lid % ST
        f = pos // 128
        rr = pos % 128
        g8 = rr // 16
        cs = rr % 16
        j = f * 16 + cs
        k = j // CH
        q = j % CH
        nidx = t * ST + (8 * k + g8) * CH + q       # into [NST, 56, CH] flat
        # lookup column within the supertile's gather calls, per stream:
        # call A (chunk-pairs 0..3): src 512k+q, dst 2048+512k+q
        # call B (chunk-pairs 4..6): src 512(k-4)+q, dst 1536+512(k-4)+q
        inA = k < 4
        iA = np.where(inA, CH * k + q, CH * (k - 4) + q)
        span = np.where(inA, 4 * CH, 3 * CH)        # per-stream width of call
        cbase = np.where(inA, 0, 8 * CH)            # call base in cf cols
        wbase = np.where(inA, 0, CH // 2)           # call base in wi cols
        _maps = (nidx, t, rr, g8, iA, span, cbase, wbase)
    return _maps


def _stream_idx(s):
    """(widx, cidx) flat positions for stream s in wi [NST,128,IDXF] and
    cf [NST,8,WO2]."""
    nidx, t, rr, g8, iA, span, cbase, wbase = _get_maps()
    i = iA + span * s
    cidx = (t * 8 + g8) * WO2 + cbase + i
    widx = (t * 128 + rr) * IDXF + wbase + i // 16
    return widx, cidx


def prepare_in_maps(embedding, edges, noise, W_emb, b_emb, W_edge, b_edge):
    import ml_dtypes
    embedding = np.asarray(embedding, dtype=np.float32)
    edges = np.asarray(edges)
    noise = np.asarray(noise, dtype=np.float32)
    W_emb = np.asarray(W_emb, dtype=np.float32)
    b_emb = np.asarray(b_emb, dtype=np.float32)
    W_edge = np.asarray(W_edge, dtype=np.float32)
    b_edge = np.float32(b_edge)

    bf = ml_dtypes.bfloat16
    wbar = ((W_edge[:HID] + W_edge[HID:]) * 0.5).astype(np.float32)
    # w01[p, s*HID+h] = W_emb[s*128+p, h]
    w01 = np.ascontiguousarray(
        W_emb.reshape(2, 128, HID).transpose(1, 0, 2).reshape(128, 2 * HID)
    ).astype(bf)
    p = np.arange(128)
    ex8 = (p[None, :] // 16 == np.arange(8)[:, None]).astype(bf)

    a1, b1 = 2.0 * BIAS - 1.0, 1.0 - BIAS
    a2, b2 = 1.0 - 2.0 * BIAS, BIAS
    cb = np.zeros((128, CBW + 128), dtype=np.float32)
    cb[:, CBW:CBW + 64] = np.ascontiguousarray(w01).view(np.float32)
    cb[0:8, CBW + 64:CBW + 128] = np.ascontiguousarray(ex8).view(np.float32)
    cb[:, CB_BD + 24:CB_BD + 32] = (p[:, None] // 16 ==
                                    np.arange(8)[None, :]).astype(np.float32)
    cb[:, CB_IOTA] = (p % 16).astype(np.float32)
    cb[0:HID, CB_BEMB] = b_emb
    cb[0:HID, CB_WBAR] = wbar
    cb[0, CB_BHALF] = b_edge * 0.5
    cb[0:56, CB_I56:CB_I56 + 56] = np.eye(56, dtype=np.float32)
    cb[0:56, CB_A1] = a1
    cb[0:56, CB_B1] = b1
    cb[0:56, CB_A2] = a2
    cb[0:56, CB_B2] = b2

    cbr = np.zeros((128, 169), dtype=np.float32)
    cbr[0:56, 113:169] = -np.eye(56, dtype=np.float32)
    cbr[:, 0:56] = cb[:, CB_BD:CB_BD + 56]
    cbr[0:56, 56:112] = cb[0:56, CB_I56:CB_I56 + 56]
    cbr[0:HID, 112] = wbar

    nidx = _get_maps()[0]
    sidx = [_stream_idx(0), _stream_idx(1)]

    in_maps = []
    for core in range(NCORES):
        sl = embedding[core * NNC:(core + 1) * NNC]
        embT = np.zeros((128, 2, NNCP), dtype=bf)
        embT[:, 0, :NNC] = sl.T[:128].astype(bf)
        embT[:, 1, :NNC] = sl.T[128:].astype(bf)

        wi = np.zeros(NST * 128 * IDXF, dtype=np.int16)
        cf = np.zeros(NST * 8 * WO2, dtype=np.float32)
        for s in range(2):
            n = edges[s, core * EC:(core + 1) * EC].astype(np.int64)
            kk = n // NNC
            npr = NNCP * kk + (n - NNC * kk)
            c = npr // TABW
            w = npr - TABW * c
            widx, cidx = sidx[s]
            wi[widx] = w.astype(np.int16)
            cf[cidx] = c.astype(np.float32)
        nz = np.full(NST * 56 * CH, 0.5, dtype=np.float32)
        nz[nidx] = noise[core * EC:(core + 1) * EC]

        in_maps.append({
            "embT": embT,
            "wi": wi.reshape(NST, 128, IDXF),
            "cf": cf.reshape(NST, 8, WO2).astype(bf),
            "nz": nz.reshape(NST, 56, CH),
            "cb": cb, "cbr": cbr,
        })
    return in_maps


def kernel(embedding, edges, noise, W_emb, b_emb, W_edge, b_edge):
    from concourse import bass_utils
    nc = _get_nc()
    in_maps = prepare_in_maps(embedding, edges, noise, W_emb, b_emb,
                              W_edge, b_edge)
    res = bass_utils.run_bass_kernel_spmd(nc, in_maps,
                                          core_ids=list(range(NCORES)))
    nidx = _get_maps()[0]
    out = np.empty(E, dtype=np.float32)
    for core in range(NCORES):
        out[core * EC:(core + 1) * EC] = \
            res.results[core]["out"].reshape(-1)[nidx]
    return out
